# revision 2
# baseline (speedup 1.0000x reference)
"""Trainium2 Bass kernel for the Black_oil loss function (approach==1 branch).

Contract: kernel(**inputs) takes the FULL inputs (shapes hardcoded below),
shards batch B=16 across 8 NeuronCores (2 batches per core, data parallel,
no communication), runs one SPMD Bass program via run_bass_kernel_spmd,
and returns the full (p_loss, s_loss) tuple of float32 arrays.

v4 design notes (v3 measured: DVE 100us busy / ScalarE 98us busy / 112us
span -> both vector engines co-saturated; all deltas below attack busy time
and queue overhead, not modes -- TT is already 2x_1p, ScalarE is always 1x):
 - fp16 at the HBM boundary both directions (from v3): host pre-casts and
   pre-transposes pressure/prior-sat to [b, x, t, y] fp16; outputs fp16
   [b, x, t, y], upcast/transposed on host.
 - Sign restructure kills two per-chunk ops: posot = [W|W] + [m2*C | mw'*C]
   with mw' = Mw/kr, both halves ADD, so no -kr*W tensor_scalar and no
   negation in the m2 Copy (zcoef0 = msqt + gam, scale=+1). The host folds
   s_loss = -kr * s~ into the fp32 upcast (scalar constant, pure unshard
   postprocessing like the existing transpose/cast).
 - Paired zu: ONE tensor_tensor [NX,2,SUP,NY] = zcoef (.) [C|C] (C broadcast
   by stride-0 dim) replaces zu0+zu1. mmc is plane-contiguous [NX,2,SUP,NY]
   ([Dx | DD]) so acm is ONE TT with a [px|a2] pair tile broadcast over t.
 - ScalarE PSUM->SBUF copies at 8-t granularity ([NX,2,8,NY] fp32 PSUM pair
   tiles = 4 banks, double buffered = all 8 banks): half the copy instrs.
 - SUPS=(12,20,20,8): fewer chunks -> fewer instrs+semaphores on the two
   saturated queues; small last chunk shortens the drain tail.
 - Output DMAs split across queues: pl via gpsimd (SWDGE, idle queue),
   sl via sync. Input/const DMAs stay on sync/scalar HWDGE.
 - NO GpSimd elementwise work (SBUF port conflict slows DVE TTs 3.6x) and
   NO scalar_tensor_tensor (measured 4x slower than TT) -- both from v2.

Math (scalar constants folded on host):
  q = prior sat ; S = 1.25q - 0.125 ; msqt = (ms*q+mb)^2 = Mw+Mo-gam
  W  = px (.) Dx(p) + py (.) Dy(p)     (px/py carry c1*64^2*600*500*k_a1)
  C  = a2 (.) DD5(p)                   (a2 carries c1*128^2*600*500)
  p_loss = W + (msqt+gam) (.) C        (F1 source term ~1e-6 rel: dropped)
  s~     = W + (Mw/kr) (.) C ; s_loss = -kr * s~   (host scales)
"""

import numpy as np

import concourse.bass as bass
import concourse.tile as tile
from concourse import bacc, mybir
from concourse.bass_utils import run_bass_kernel_spmd

B, T, NX, NY = 16, 60, 128, 128
NCORES = 8
BPC = B // NCORES   # batches per core
SUPS = (12, 20, 20, 8)   # small first chunk = short fill; small last = tail
SUPMAX = max(SUPS)
SUB = 4             # t per matmul (one 512-elem PSUM bank slice)
SUBC = 8            # t per PSUM pair-tile / ScalarE copy (4 banks)

# reference constants
UIR = 5000.0; PINI_ALT = 600.0; LUB = 0.1; HUB = 1.0; AAY = 50.0; BBY = 500.0
SWI = 0.1; SWR = 0.1; UW = 1.0; BW = 1.0; UO = 2.5; BO = 1.1; MAXZ = 6000.0

F32 = mybir.dt.float32
F16 = mybir.dt.float16
OP = mybir.AluOpType
ACTF = mybir.ActivationFunctionType


def _consts(siniuse):
    dxf = 1.0 / NY
    c1 = dxf * 1e-7
    m_r = (BBY - AAY) / (HUB - LUB)
    b_r = AAY - m_r * LUB
    s0 = (siniuse - SWI) / (1.0 - SWI - SWR)
    k_w = s0 * s0 / (UW * BW)
    k_a1 = k_w + (1.0 - s0) ** 2 / (UO * BO)
    kr = k_w / k_a1
    cpx = c1 * 64.0 * 64.0 * PINI_ALT * m_r * k_a1   # k_a1 folded into W
    cdd = c1 * 16384.0 * PINI_ALT

    # complete the square: Mw + Mo = (msq_scale*q + msq_bias)^2 + gam
    iuo = 1.0 / (UO * BO)
    a1c = 1.0 + iuo
    sst = iuo / a1c
    gam = sst * sst + (1.0 - sst) ** 2 * iuo
    ra = a1c ** 0.5
    msq_scale = ra * 1.25
    msq_bias = ra * (-0.125 - sst)
    # Mw/kr = ((1.25q - 0.125)/sqrt(kr))^2
    rkr = kr ** -0.5
    mw_scale = 1.25 * rkr
    mw_bias = -0.125 * rkr
    return dict(m_r=m_r, b_r=b_r, kr=kr, cpx=cpx, cdd=cdd, gam=gam,
                msq_scale=msq_scale, msq_bias=msq_bias,
                mw_scale=mw_scale, mw_bias=mw_bias)


def _stencil_mats():
    d1 = np.zeros((NX, NX), np.float64)
    d2 = np.zeros((NX, NX), np.float64)
    for m in range(NX):
        d1[m, min(m + 1, NX - 1)] += 1.0
        d1[m, max(m - 1, 0)] -= 1.0
        d2[m, min(m + 1, NX - 1)] += 1.0
        d2[m, max(m - 1, 0)] += 1.0
        d2[m, m] -= 2.0
    d2m = d2 - 2.0 * np.eye(NX)  # fold the y-second-diff -2u term
    return (np.ascontiguousarray(d1.T, np.float16),
            np.ascontiguousarray(d2m.T, np.float16))


def _build(siniuse):
    cc = _consts(siniuse)
    assert sum(SUPS) == T and all(s % SUB == 0 for s in SUPS)

    nc = bacc.Bacc("TRN2", target_bir_lowering=False, debug=False,
                   num_devices=NCORES)
    p_in = nc.dram_tensor("p", [BPC, NX, T, NY + 2], F16,
                          kind="ExternalInput").ap()
    q_in = nc.dram_tensor("q", [BPC, NX, T, NY], F16,
                          kind="ExternalInput").ap()
    perm_in = nc.dram_tensor("permp", [NX, BPC, NY + 2], F16,
                             kind="ExternalInput").ap()
    d1_in = nc.dram_tensor("d1t", [NX, NX], F16, kind="ExternalInput").ap()
    d2_in = nc.dram_tensor("d2t", [NX, NX], F16, kind="ExternalInput").ap()
    id_in = nc.dram_tensor("ident", [NX, NX], F16, kind="ExternalInput").ap()
    pl = nc.dram_tensor("p_loss", [BPC, NX, T, NY], F16,
                        kind="ExternalOutput").ap()
    sl = nc.dram_tensor("s_loss", [BPC, NX, T, NY], F16,
                        kind="ExternalOutput").ap()

    bw = BPC * NY

    with tile.TileContext(nc) as tc:
        with tc.tile_pool(name="const", bufs=1) as cp:
            # sync queue: permp (gates DVE preproc) then d1t (preproc
            # matmul) lead; d2t/idt ride the scalar queue (its squares wait
            # on qt DMAs anyway)
            permp = cp.tile([NX, BPC, NY + 2], F16)
            nc.sync.dma_start(permp[:], perm_in[:, :, :])
            d1t = cp.tile([NX, NX], F16)
            nc.sync.dma_start(d1t[:], d1_in[:, :])
            d2t = cp.tile([NX, NX], F16)
            nc.scalar.dma_start(d2t[:], d2_in[:, :])
            idt = cp.tile([NX, NX], F16)
            nc.scalar.dma_start(idt[:], id_in[:, :])

            b_mw = cp.tile([NX, 1], F32)
            nc.vector.memset(b_mw[:], cc["mw_bias"])
            b_msq = cp.tile([NX, 1], F32)
            nc.vector.memset(b_msq[:], cc["msq_bias"])

            # ---- per-batch small-tile preprocessing (one-time) ----
            # py2 first (needs only permp; gates first chunk's btile);
            # pxa2 = [px | a2] pair tile (px via PE matmul path)
            py2 = cp.tile([NX, bw], F16)
            pxa2 = cp.tile([NX, BPC, 2, NY], F16)

            rdyp = cp.tile([NX, bw], F16)
            nc.vector.tensor_tensor(
                rdyp[:].rearrange("p (b y) -> p b y", b=BPC),
                permp[:, :, 2:NY + 2], permp[:, :, 0:NY], OP.subtract)
            nc.vector.tensor_scalar(py2[:], rdyp[:], cc["cpx"], None, OP.mult)
            nc.vector.tensor_scalar(
                pxa2[:, :, 1, :], permp[:, :, 1:NY + 1],
                cc["cdd"] * cc["m_r"], cc["cdd"] * cc["b_r"], OP.mult, OP.add)

            with tc.tile_pool(name="ppsum", bufs=1, space="PSUM") as pp:
                mmp = pp.tile([NX, bw], F32)
                nc.tensor.matmul(
                    mmp[:].rearrange("p (b y) -> p b y", b=BPC),
                    d1t[:], permp[:, :, 1:NY + 1], start=True, stop=True)
                nc.vector.tensor_scalar(
                    pxa2[:, :, 0, :],
                    mmp[:].rearrange("p (b y) -> p b y", b=BPC),
                    cc["cpx"], None, OP.mult)

            # ---- main loop over (super-chunk, batch) ----
            with tc.tile_pool(name="deep", bufs=3) as dp_, \
                 tc.tile_pool(name="sup", bufs=2) as sp_, \
                 tc.tile_pool(name="mmpool", bufs=2, space="PSUM") as mp:
                t0s = []
                acc = 0
                for SUP in SUPS:
                    t0s.append(acc)
                    acc += SUP
                # interleave batches: consecutive chunks are independent,
                # giving the static schedule slack to overlap
                for sc, SUP in enumerate(SUPS):
                    for b in range(BPC):
                        t0 = t0s[sc]
                        pt = dp_.tile([NX, SUPMAX, NY + 2], F16, tag="pt")
                        nc.sync.dma_start(pt[:, 0:SUP, :],
                                          p_in[b, :, t0:t0 + SUP, :])
                        qt = sp_.tile([NX, SUPMAX, NY], F16, tag="qt")
                        nc.sync.dma_start(qt[:, 0:SUP, :],
                                          q_in[b, :, t0:t0 + SUP, :])

                        # coefficient fields for the zu pair (ScalarE).
                        # zcoef plane1 = Mw/kr (Square, scaled); msqt ->
                        # plane0 = msqt + gam (Copy) AFTER the PSUM copies
                        # (msq2's consumer zupair runs late in the chain)
                        zcoef = sp_.tile([NX, 2, SUPMAX, NY], F16,
                                         tag="zcoef")
                        nc.scalar.activation(zcoef[:, 1, 0:SUP, :],
                                             qt[:, 0:SUP, :], ACTF.Square,
                                             bias=b_mw[:],
                                             scale=cc["mw_scale"])
                        msqt = sp_.tile([NX, SUPMAX, NY], F16, tag="msqt")
                        nc.scalar.activation(msqt[:, 0:SUP, :],
                                             qt[:, 0:SUP, :], ACTF.Square,
                                             bias=b_msq[:],
                                             scale=cc["msq_scale"])

                        # stencil matmuls into [Dx|DD] PSUM pair tiles at
                        # SUBC=8 granularity; ONE ScalarE copy per group
                        # into the plane-contiguous fp16 mmc
                        mmc = dp_.tile([NX, 2, SUPMAX, NY], F16, tag="mmc")
                        for goff in range(0, SUP, SUBC):
                            h = min(SUBC, SUP - goff)
                            mm = mp.tile([NX, 2, SUBC, NY], F32, tag="mm")
                            for l in range(0, h, SUB):
                                pv = pt[:, goff + l:goff + l + SUB, :]
                                sl_ = slice(l, l + SUB)
                                nc.tensor.matmul(mm[:, 0, sl_, :], d1t[:],
                                                 pv[:, :, 1:NY + 1],
                                                 start=True, stop=True)
                                nc.tensor.matmul(mm[:, 1, sl_, :], d2t[:],
                                                 pv[:, :, 1:NY + 1],
                                                 start=True, stop=False)
                                nc.tensor.matmul(mm[:, 1, sl_, :], idt[:],
                                                 pv[:, :, 2:NY + 2],
                                                 start=False, stop=False)
                                nc.tensor.matmul(mm[:, 1, sl_, :], idt[:],
                                                 pv[:, :, 0:NY],
                                                 start=False, stop=True)
                            nc.scalar.copy(mmc[:, :, goff:goff + h, :],
                                           mm[:, :, 0:h, :])

                        # msq2 -> zcoef plane0 (ScalarE, after the copies)
                        nc.scalar.activation(zcoef[:, 0, 0:SUP, :],
                                             msqt[:, 0:SUP, :], ACTF.Copy,
                                             bias=cc["gam"], scale=1.0)

                        # ---- DVE chain ----
                        rawdy = sp_.tile([NX, SUPMAX, NY], F16, tag="rawdy")
                        nc.vector.tensor_tensor(
                            rawdy[:, 0:SUP, :], pt[:, 0:SUP, 2:NY + 2],
                            pt[:, 0:SUP, 0:NY], OP.subtract)
                        btile = sp_.tile([NX, SUPMAX, NY], F16, tag="bt")
                        nc.vector.tensor_tensor(
                            btile[:, 0:SUP, :],
                            py2[:, b * NY:(b + 1) * NY].unsqueeze(1)
                            .broadcast_to([NX, SUP, NY]),
                            rawdy[:, 0:SUP, :], OP.mult)

                        # A and C in ONE tensor_tensor over the mmc planes
                        acm = sp_.tile([NX, 2, SUPMAX, NY], F16, tag="acm")
                        nc.vector.tensor_tensor(
                            acm[:, :, 0:SUP, :],
                            pxa2[:, b, :, :].unsqueeze(2)
                            .broadcast_to([NX, 2, SUP, NY]),
                            mmc[:, :, 0:SUP, :], OP.mult)

                        wt = sp_.tile([NX, SUPMAX, NY], F16, tag="wt")
                        nc.vector.tensor_tensor(
                            wt[:, 0:SUP, :], acm[:, 0, 0:SUP, :],
                            btile[:, 0:SUP, :], OP.add)

                        # zu pair = zcoef (.) [C|C] (C broadcast, stride-0)
                        zu = sp_.tile([NX, 2, SUPMAX, NY], F16, tag="zu")
                        nc.vector.tensor_tensor(
                            zu[:, :, 0:SUP, :], zcoef[:, :, 0:SUP, :],
                            acm[:, 1, 0:SUP, :].unsqueeze(1)
                            .broadcast_to([NX, 2, SUP, NY]), OP.mult)

                        # [p | s~] = [W|W] + zu; host applies s = -kr*s~
                        posot = sp_.tile([NX, 2, SUPMAX, NY], F16, tag="po")
                        last = (b == BPC - 1 and sc == len(SUPS) - 1)
                        hs = [(0, SUP)] if not last else \
                            [(k, k + SUB) for k in range(0, SUP, SUB)]
                        for (ha, hb) in hs:
                            nc.vector.tensor_tensor(
                                posot[:, :, ha:hb, :],
                                wt[:, ha:hb, :].unsqueeze(1)
                                .broadcast_to([NX, 2, hb - ha, NY]),
                                zu[:, :, ha:hb, :], OP.add)
                            nc.gpsimd.dma_start(
                                pl[b, :, t0 + ha:t0 + hb, :],
                                posot[:, 0, ha:hb, :])
                            nc.sync.dma_start(
                                sl[b, :, t0 + ha:t0 + hb, :],
                                posot[:, 1, ha:hb, :])
    nc.compile()
    return nc


_CACHE = {}

TRACE = False
LAST_RESULT = None


def _get_program(siniuse):
    key = (float(siniuse), T, SUPS, SUB, SUBC)
    if key not in _CACHE:
        _CACHE[key] = _build(float(siniuse))
    return _CACHE[key]


def kernel(pressure, perm, Q, Qw, Time, Pini, Phi, Swini, water_sat):
    pressure = np.asarray(pressure, np.float32)
    water_sat = np.asarray(water_sat, np.float32)
    perm = np.asarray(perm, np.float32)
    Swini = np.asarray(Swini, np.float32)

    siniuse = float(Swini[0, 0, 0, 0])
    nc = _get_program(siniuse)
    cc = _consts(siniuse)
    d1t, d2t = _stencil_mats()
    ident = np.eye(NX, dtype=np.float16)

    # host-side layout/dtype prep (pure data movement, no arithmetic)
    pr_t = np.ascontiguousarray(pressure.transpose(0, 2, 1, 3))
    pr_pad = np.empty((B, NX, T, NY + 2), np.float16)
    pr_pad[:, :, :, 1:NY + 1] = pr_t
    pr_pad[:, :, :, 0] = pr_t[:, :, :, 0]
    pr_pad[:, :, :, NY + 1] = pr_t[:, :, :, NY - 1]
    prior = np.empty((B, NX, T, NY), np.float16)
    prior[:, :, 0, :] = np.float16(siniuse)
    prior[:, :, 1:, :] = water_sat[:, :T - 1].transpose(0, 2, 1, 3)
    pm_t = perm[:, 0].transpose(1, 0, 2)  # [X, B, Y]
    pm_pad = np.empty((NX, B, NY + 2), np.float16)
    pm_pad[:, :, 1:NY + 1] = pm_t
    pm_pad[:, :, 0] = pm_t[:, :, 0]
    pm_pad[:, :, NY + 1] = pm_t[:, :, NY - 1]

    expected = set()
    for alloc in nc.m.functions[0].allocations:
        if getattr(alloc, "kind", None) == "ExternalInput":
            expected.add(alloc.memorylocations[0].name)

    in_maps = []
    for c in range(NCORES):
        s = slice(c * BPC, (c + 1) * BPC)
        full = {
            "p": np.ascontiguousarray(pr_pad[s]),
            "q": np.ascontiguousarray(prior[s]),
            "permp": np.ascontiguousarray(pm_pad[:, s]),
            "d1t": d1t,
            "d2t": d2t,
            "ident": ident,
        }
        in_maps.append({k: v for k, v in full.items() if k in expected})

    res = run_bass_kernel_spmd(nc, in_maps, core_ids=list(range(NCORES)),
                               trace=TRACE)
    global LAST_RESULT
    LAST_RESULT = res
    p_loss = np.concatenate(
        [res.results[c]["p_loss"] for c in range(NCORES)], axis=0)
    s_loss = np.concatenate(
        [res.results[c]["s_loss"] for c in range(NCORES)], axis=0)
    p_loss = np.ascontiguousarray(
        p_loss.astype(np.float32).transpose(0, 2, 1, 3))
    s_loss = np.ascontiguousarray(
        s_loss.astype(np.float32).transpose(0, 2, 1, 3) *
        np.float32(-cc["kr"]))
    return p_loss, s_loss


# revision 3
# speedup vs baseline: 1.1512x; 1.1512x over previous
"""Trainium2 Bass kernel for the Black_oil loss function (approach==1 branch).

Contract: kernel(**inputs) takes the FULL inputs (shapes hardcoded below),
shards batch B=16 across 8 NeuronCores (2 batches per core, data parallel,
no communication), runs one SPMD Bass program via run_bass_kernel_spmd,
and returns the full (p_loss, s_loss) tuple of float32 arrays.

v4 design notes (v3 measured: DVE 100us busy / ScalarE 98us busy / 112us
span -> both vector engines co-saturated; all deltas below attack busy time
and queue overhead, not modes -- TT is already 2x_1p, ScalarE is always 1x):
 - fp16 at the HBM boundary both directions (from v3): host pre-casts and
   pre-transposes pressure/prior-sat to [b, x, t, y] fp16; outputs fp16
   [b, x, t, y], upcast/transposed on host.
 - Sign restructure kills two per-chunk ops: posot = [W|W] + [m2*C | mw'*C]
   with mw' = Mw/kr, both halves ADD, so no -kr*W tensor_scalar and no
   negation in the m2 Copy (zcoef0 = msqt + gam, scale=+1). The host folds
   s_loss = -kr * s~ into the fp32 upcast (scalar constant, pure unshard
   postprocessing like the existing transpose/cast).
 - Paired zu: ONE tensor_tensor [NX,2,SUP,NY] = zcoef (.) [C|C] (C broadcast
   by stride-0 dim) replaces zu0+zu1. mmc is plane-contiguous [NX,2,SUP,NY]
   ([Dx | DD]) so acm is ONE TT with a [px|a2] pair tile broadcast over t.
 - ScalarE PSUM->SBUF copies at 8-t granularity ([NX,2,8,NY] fp32 PSUM pair
   tiles = 4 banks, double buffered = all 8 banks): half the copy instrs.
 - SUPS=(12,20,20,8): fewer chunks -> fewer instrs+semaphores on the two
   saturated queues; small last chunk shortens the drain tail.
 - Output DMAs split across queues: pl via gpsimd (SWDGE, idle queue),
   sl via sync. Input/const DMAs stay on sync/scalar HWDGE.
 - NO GpSimd elementwise work (SBUF port conflict slows DVE TTs 3.6x) and
   NO scalar_tensor_tensor (measured 4x slower than TT) -- both from v2.

Math (scalar constants folded on host):
  q = prior sat ; S = 1.25q - 0.125 ; msqt = (ms*q+mb)^2 = Mw+Mo-gam
  W  = px (.) Dx(p) + py (.) Dy(p)     (px/py carry c1*64^2*600*500*k_a1)
  C  = a2 (.) DD5(p)                   (a2 carries c1*128^2*600*500)
  p_loss = W + (msqt+gam) (.) C        (F1 source term ~1e-6 rel: dropped)
  s~     = W + (Mw/kr) (.) C ; s_loss = -kr * s~   (host scales)
"""

import numpy as np

import concourse.bass as bass
import concourse.tile as tile
from concourse import bacc, mybir
from concourse.bass_utils import run_bass_kernel_spmd

B, T, NX, NY = 16, 60, 128, 128
NCORES = 8
BPC = B // NCORES   # batches per core
SUPS = (12, 20, 20, 8)   # small first chunk = short fill; small last = tail
SUPMAX = max(SUPS)
SUB = 4             # t per matmul / PSUM pair tile (2 banks) / ScalarE copy

# reference constants
UIR = 5000.0; PINI_ALT = 600.0; LUB = 0.1; HUB = 1.0; AAY = 50.0; BBY = 500.0
SWI = 0.1; SWR = 0.1; UW = 1.0; BW = 1.0; UO = 2.5; BO = 1.1; MAXZ = 6000.0

F32 = mybir.dt.float32
F16 = mybir.dt.float16
OP = mybir.AluOpType
ACTF = mybir.ActivationFunctionType


def _consts(siniuse):
    dxf = 1.0 / NY
    c1 = dxf * 1e-7
    m_r = (BBY - AAY) / (HUB - LUB)
    b_r = AAY - m_r * LUB
    s0 = (siniuse - SWI) / (1.0 - SWI - SWR)
    k_w = s0 * s0 / (UW * BW)
    k_a1 = k_w + (1.0 - s0) ** 2 / (UO * BO)
    kr = k_w / k_a1
    cpx = c1 * 64.0 * 64.0 * PINI_ALT * m_r * k_a1   # k_a1 folded into W
    cdd = c1 * 16384.0 * PINI_ALT

    # complete the square: Mw + Mo = (msq_scale*q + msq_bias)^2 + gam
    iuo = 1.0 / (UO * BO)
    a1c = 1.0 + iuo
    sst = iuo / a1c
    gam = sst * sst + (1.0 - sst) ** 2 * iuo
    ra = a1c ** 0.5
    msq_scale = ra * 1.25
    msq_bias = ra * (-0.125 - sst)
    # Mw/kr = ((1.25q - 0.125)/sqrt(kr))^2
    rkr = kr ** -0.5
    mw_scale = 1.25 * rkr
    mw_bias = -0.125 * rkr
    return dict(m_r=m_r, b_r=b_r, kr=kr, cpx=cpx, cdd=cdd, gam=gam,
                msq_scale=msq_scale, msq_bias=msq_bias,
                mw_scale=mw_scale, mw_bias=mw_bias)


def _stencil_mats():
    d1 = np.zeros((NX, NX), np.float64)
    d2 = np.zeros((NX, NX), np.float64)
    for m in range(NX):
        d1[m, min(m + 1, NX - 1)] += 1.0
        d1[m, max(m - 1, 0)] -= 1.0
        d2[m, min(m + 1, NX - 1)] += 1.0
        d2[m, max(m - 1, 0)] += 1.0
        d2[m, m] -= 2.0
    d2m = d2 - 2.0 * np.eye(NX)  # fold the y-second-diff -2u term
    return (np.ascontiguousarray(d1.T, np.float16),
            np.ascontiguousarray(d2m.T, np.float16))


def _build(siniuse):
    cc = _consts(siniuse)
    assert sum(SUPS) == T and all(s % SUB == 0 for s in SUPS)

    nc = bacc.Bacc("TRN2", target_bir_lowering=False, debug=False,
                   num_devices=NCORES)
    p_in = nc.dram_tensor("p", [BPC, NX, T, NY + 2], F16,
                          kind="ExternalInput").ap()
    q_in = nc.dram_tensor("q", [BPC, NX, T, NY], F16,
                          kind="ExternalInput").ap()
    perm_in = nc.dram_tensor("permp", [NX, BPC, NY + 2], F16,
                             kind="ExternalInput").ap()
    d1_in = nc.dram_tensor("d1t", [NX, NX], F16, kind="ExternalInput").ap()
    d2_in = nc.dram_tensor("d2t", [NX, NX], F16, kind="ExternalInput").ap()
    id_in = nc.dram_tensor("ident", [NX, NX], F16, kind="ExternalInput").ap()
    pl = nc.dram_tensor("p_loss", [BPC, NX, T, NY], F16,
                        kind="ExternalOutput").ap()
    sl = nc.dram_tensor("s_loss", [BPC, NX, T, NY], F16,
                        kind="ExternalOutput").ap()

    bw = BPC * NY

    with tile.TileContext(nc) as tc:
        with tc.tile_pool(name="const", bufs=1) as cp:
            # sync queue: permp (gates DVE preproc) then d1t (preproc
            # matmul) lead; d2t/idt ride the scalar queue (its squares wait
            # on qt DMAs anyway)
            permp = cp.tile([NX, BPC, NY + 2], F16)
            nc.sync.dma_start(permp[:], perm_in[:, :, :])
            d1t = cp.tile([NX, NX], F16)
            nc.sync.dma_start(d1t[:], d1_in[:, :])
            d2t = cp.tile([NX, NX], F16)
            nc.scalar.dma_start(d2t[:], d2_in[:, :])
            idt = cp.tile([NX, NX], F16)
            nc.scalar.dma_start(idt[:], id_in[:, :])

            b_mw = cp.tile([NX, 1], F32)
            nc.vector.memset(b_mw[:], cc["mw_bias"])
            b_msq = cp.tile([NX, 1], F32)
            nc.vector.memset(b_msq[:], cc["msq_bias"])

            # ---- per-batch small-tile preprocessing (one-time) ----
            # py2 first (needs only permp; gates first chunk's btile);
            # pxa2 = [px | a2] pair tile (px via PE matmul path)
            py2 = cp.tile([NX, bw], F16)
            pxa2 = cp.tile([NX, BPC, 2, NY], F16)

            rdyp = cp.tile([NX, bw], F16)
            nc.vector.tensor_tensor(
                rdyp[:].rearrange("p (b y) -> p b y", b=BPC),
                permp[:, :, 2:NY + 2], permp[:, :, 0:NY], OP.subtract)
            nc.vector.tensor_scalar(py2[:], rdyp[:], cc["cpx"], None, OP.mult)
            nc.vector.tensor_scalar(
                pxa2[:, :, 1, :], permp[:, :, 1:NY + 1],
                cc["cdd"] * cc["m_r"], cc["cdd"] * cc["b_r"], OP.mult, OP.add)

            with tc.tile_pool(name="ppsum", bufs=1, space="PSUM") as pp:
                mmp = pp.tile([NX, bw], F32)
                nc.tensor.matmul(
                    mmp[:].rearrange("p (b y) -> p b y", b=BPC),
                    d1t[:], permp[:, :, 1:NY + 1], start=True, stop=True)
                nc.vector.tensor_scalar(
                    pxa2[:, :, 0, :],
                    mmp[:].rearrange("p (b y) -> p b y", b=BPC),
                    cc["cpx"], None, OP.mult)

            # ---- main loop over (super-chunk, batch) ----
            with tc.tile_pool(name="deep", bufs=3) as dp_, \
                 tc.tile_pool(name="sup", bufs=2) as sp_, \
                 tc.tile_pool(name="zpool", bufs=3) as zp_, \
                 tc.tile_pool(name="mmpool", bufs=4, space="PSUM") as mp:
                t0s = []
                acc = 0
                for SUP in SUPS:
                    t0s.append(acc)
                    acc += SUP
                # interleave batches: consecutive chunks are independent,
                # giving the static schedule slack to overlap
                for sc, SUP in enumerate(SUPS):
                    for b in range(BPC):
                        t0 = t0s[sc]
                        pt = dp_.tile([NX, SUPMAX, NY + 2], F16, tag="pt")
                        nc.sync.dma_start(pt[:, 0:SUP, :],
                                          p_in[b, :, t0:t0 + SUP, :])
                        qt = sp_.tile([NX, SUPMAX, NY], F16, tag="qt")
                        nc.sync.dma_start(qt[:, 0:SUP, :],
                                          q_in[b, :, t0:t0 + SUP, :])

                        # coefficient fields for the zu pair (ScalarE).
                        # zcoef plane1 = Mw/kr (Square, scaled); msqt ->
                        # plane0 = msqt + gam (Copy) AFTER the PSUM copies
                        # (msq2's consumer zupair runs late in the chain)
                        zcoef = zp_.tile([NX, 2, SUPMAX, NY], F16,
                                         tag="zcoef")
                        nc.scalar.activation(zcoef[:, 1, 0:SUP, :],
                                             qt[:, 0:SUP, :], ACTF.Square,
                                             bias=b_mw[:],
                                             scale=cc["mw_scale"])
                        msqt = sp_.tile([NX, SUPMAX, NY], F16, tag="msqt")
                        nc.scalar.activation(msqt[:, 0:SUP, :],
                                             qt[:, 0:SUP, :], ACTF.Square,
                                             bias=b_msq[:],
                                             scale=cc["msq_scale"])

                        # stencil matmuls into [Dx|DD] PSUM pair tiles
                        # (2 banks each, quad-buffered so the PE never
                        # stalls on ScalarE draining); one ScalarE copy per
                        # SUB=4 group into the plane-contiguous fp16 mmc
                        mmc = dp_.tile([NX, 2, SUPMAX, NY], F16, tag="mmc")
                        for goff in range(0, SUP, SUB):
                            pv = pt[:, goff:goff + SUB, :]
                            mm = mp.tile([NX, 2, SUB, NY], F32, tag="mm")
                            nc.tensor.matmul(mm[:, 0, :, :], d1t[:],
                                             pv[:, :, 1:NY + 1],
                                             start=True, stop=True)
                            nc.tensor.matmul(mm[:, 1, :, :], d2t[:],
                                             pv[:, :, 1:NY + 1],
                                             start=True, stop=False)
                            nc.tensor.matmul(mm[:, 1, :, :], idt[:],
                                             pv[:, :, 2:NY + 2],
                                             start=False, stop=False)
                            nc.tensor.matmul(mm[:, 1, :, :], idt[:],
                                             pv[:, :, 0:NY],
                                             start=False, stop=True)
                            nc.scalar.copy(mmc[:, :, goff:goff + SUB, :],
                                           mm[:, :, :, :])

                        # msq2 -> zcoef plane0 (ScalarE, after the copies)
                        nc.scalar.activation(zcoef[:, 0, 0:SUP, :],
                                             msqt[:, 0:SUP, :], ACTF.Copy,
                                             bias=cc["gam"], scale=1.0)

                        # ---- DVE chain ----
                        rawdy = sp_.tile([NX, SUPMAX, NY], F16, tag="rawdy")
                        nc.vector.tensor_tensor(
                            rawdy[:, 0:SUP, :], pt[:, 0:SUP, 2:NY + 2],
                            pt[:, 0:SUP, 0:NY], OP.subtract)
                        btile = sp_.tile([NX, SUPMAX, NY], F16, tag="bt")
                        nc.vector.tensor_tensor(
                            btile[:, 0:SUP, :],
                            py2[:, b * NY:(b + 1) * NY].unsqueeze(1)
                            .broadcast_to([NX, SUP, NY]),
                            rawdy[:, 0:SUP, :], OP.mult)

                        # A and C in ONE tensor_tensor over the mmc planes
                        acm = sp_.tile([NX, 2, SUPMAX, NY], F16, tag="acm")
                        nc.vector.tensor_tensor(
                            acm[:, :, 0:SUP, :],
                            pxa2[:, b, :, :].unsqueeze(2)
                            .broadcast_to([NX, 2, SUP, NY]),
                            mmc[:, :, 0:SUP, :], OP.mult)

                        wt = sp_.tile([NX, SUPMAX, NY], F16, tag="wt")
                        nc.vector.tensor_tensor(
                            wt[:, 0:SUP, :], acm[:, 0, 0:SUP, :],
                            btile[:, 0:SUP, :], OP.add)

                        # zu pair = zcoef (.) [C|C] (C broadcast, stride-0)
                        zu = sp_.tile([NX, 2, SUPMAX, NY], F16, tag="zu")
                        nc.vector.tensor_tensor(
                            zu[:, :, 0:SUP, :], zcoef[:, :, 0:SUP, :],
                            acm[:, 1, 0:SUP, :].unsqueeze(1)
                            .broadcast_to([NX, 2, SUP, NY]), OP.mult)

                        # [p | s~] = [W|W] + zu; host applies s = -kr*s~
                        posot = sp_.tile([NX, 2, SUPMAX, NY], F16, tag="po")
                        last = (b == BPC - 1 and sc == len(SUPS) - 1)
                        hs = [(0, SUP)] if not last else \
                            [(k, k + SUB) for k in range(0, SUP, SUB)]
                        for (ha, hb) in hs:
                            nc.vector.tensor_tensor(
                                posot[:, :, ha:hb, :],
                                wt[:, ha:hb, :].unsqueeze(1)
                                .broadcast_to([NX, 2, hb - ha, NY]),
                                zu[:, :, ha:hb, :], OP.add)
                            nc.gpsimd.dma_start(
                                pl[b, :, t0 + ha:t0 + hb, :],
                                posot[:, 0, ha:hb, :])
                            nc.sync.dma_start(
                                sl[b, :, t0 + ha:t0 + hb, :],
                                posot[:, 1, ha:hb, :])
    nc.compile()
    return nc


_CACHE = {}

TRACE = False
LAST_RESULT = None


def _get_program(siniuse):
    key = (float(siniuse), T, SUPS, SUB, 5)
    if key not in _CACHE:
        _CACHE[key] = _build(float(siniuse))
    return _CACHE[key]


def kernel(pressure, perm, Q, Qw, Time, Pini, Phi, Swini, water_sat):
    pressure = np.asarray(pressure, np.float32)
    water_sat = np.asarray(water_sat, np.float32)
    perm = np.asarray(perm, np.float32)
    Swini = np.asarray(Swini, np.float32)

    siniuse = float(Swini[0, 0, 0, 0])
    nc = _get_program(siniuse)
    cc = _consts(siniuse)
    d1t, d2t = _stencil_mats()
    ident = np.eye(NX, dtype=np.float16)

    # host-side layout/dtype prep (pure data movement, no arithmetic)
    pr_t = np.ascontiguousarray(pressure.transpose(0, 2, 1, 3))
    pr_pad = np.empty((B, NX, T, NY + 2), np.float16)
    pr_pad[:, :, :, 1:NY + 1] = pr_t
    pr_pad[:, :, :, 0] = pr_t[:, :, :, 0]
    pr_pad[:, :, :, NY + 1] = pr_t[:, :, :, NY - 1]
    prior = np.empty((B, NX, T, NY), np.float16)
    prior[:, :, 0, :] = np.float16(siniuse)
    prior[:, :, 1:, :] = water_sat[:, :T - 1].transpose(0, 2, 1, 3)
    pm_t = perm[:, 0].transpose(1, 0, 2)  # [X, B, Y]
    pm_pad = np.empty((NX, B, NY + 2), np.float16)
    pm_pad[:, :, 1:NY + 1] = pm_t
    pm_pad[:, :, 0] = pm_t[:, :, 0]
    pm_pad[:, :, NY + 1] = pm_t[:, :, NY - 1]

    expected = set()
    for alloc in nc.m.functions[0].allocations:
        if getattr(alloc, "kind", None) == "ExternalInput":
            expected.add(alloc.memorylocations[0].name)

    in_maps = []
    for c in range(NCORES):
        s = slice(c * BPC, (c + 1) * BPC)
        full = {
            "p": np.ascontiguousarray(pr_pad[s]),
            "q": np.ascontiguousarray(prior[s]),
            "permp": np.ascontiguousarray(pm_pad[:, s]),
            "d1t": d1t,
            "d2t": d2t,
            "ident": ident,
        }
        in_maps.append({k: v for k, v in full.items() if k in expected})

    res = run_bass_kernel_spmd(nc, in_maps, core_ids=list(range(NCORES)),
                               trace=TRACE)
    global LAST_RESULT
    LAST_RESULT = res
    p_loss = np.concatenate(
        [res.results[c]["p_loss"] for c in range(NCORES)], axis=0)
    s_loss = np.concatenate(
        [res.results[c]["s_loss"] for c in range(NCORES)], axis=0)
    p_loss = np.ascontiguousarray(
        p_loss.astype(np.float32).transpose(0, 2, 1, 3))
    s_loss = np.ascontiguousarray(
        s_loss.astype(np.float32).transpose(0, 2, 1, 3) *
        np.float32(-cc["kr"]))
    return p_loss, s_loss
